# revision 9
# baseline (speedup 1.0000x reference)
"""Conv2d(32->32, 3x3, stride 1, pad 1) on X[32,32,224,224] fp32, data-parallel
over 8 NeuronCores (4 images per core).

Per-core algorithm ("full-K row-rotated")
-----------------------------------------
The conv is computed as full-array PE matmuls with contraction K = 128 =
(q in 0..3 row-taps) x (c = 32 input channels), M = 64 = (ho in 0..1) x
(k = 32 output channels), N = 448 = (u in 0..1 row-pairs) x (w in 0..223),
in fp16 (1 column/cycle, warms the PE clock gate; fp32 runs 4x slower and
float32r runs cold at 1.2 GHz and overlaps poorly).

X (host-padded to 226 wide, host-cast to fp16, host-rotated so row 4*jd + q
sits at partition group q) is DMA'd per H-slice as Xr0; a second copy Xr1,
rotated down by two rows (j = 4*jd + q + 2), is built on-chip by two
SBUF->SBUF partition-remap DMAs per xr0 half (engine time but no HBM
bandwidth, which the startup is short on).  For an output pair starting at
even hb0, the 4 input rows sit at partition group q, one free offset -- so
one matmul contracts all 4 row-taps at once; even pairs read Xr0, odd
pairs Xr1.  One PSUM accumulation group = 3 matmuls (s = column
shift of the rhs into the padded row).  Per core: 672 matmuls of
[128,64]x[128,448].

lhsT[32*q+c, s][32*ho+k] = W[k, c, q-ho, s] (zero outside 0<=r<3), so both
output rows of an hb are produced per matmul.  Bias is fused into the
PSUM->SBUF eviction (ScalarE/VectorE alternating), which also casts to fp16.
Work is H-sliced (112 output rows) for SBUF fit and load/compute overlap.

Y leaves the device in the staged layout [n, G, k, m, w] (fp16) where the
output row h = 4*m + G -- each store is then one >=6KB descriptor per
partition instead of 896B per (k, m) row, which otherwise saturates all 16
DMA queues with descriptor overhead.  The host un-interleaves + casts fp32.
"""

import sys

import numpy as np

try:
    import concourse.bass as bass  # noqa: F401
except ImportError:  # pragma: no cover
    sys.path.insert(0, "/opt/trn_rl_repo")

import ml_dtypes
import concourse.mybir as mybir
import concourse.tile as tile
from concourse import bacc
from concourse.bass_utils import run_bass_kernel_spmd

NCORES = 8
NB = 4  # images per core
C = 32
K = 32
H = 224
W = 224
WP = 226  # padded width
NQ = 57  # row-quads in the host-rotated layout (228 padded rows / 4)
RS = 112  # output rows per slice
NSLICE = H // RS
NJD = RS // 4 + 1  # row-quads per rotated slice tile
F32 = mybir.dt.float32
F16 = mybir.dt.float16
AF = mybir.ActivationFunctionType
_NP16 = np.float16


def set_dtype(name):
    """'fp16' (default) or 'bf16' for the matmul operand precision."""
    global F16, _NP16, _NC
    if name == "bf16":
        F16, _NP16 = mybir.dt.bfloat16, ml_dtypes.bfloat16
    else:
        F16, _NP16 = mybir.dt.float16, np.float16
    _NC = None


def conv_body(tc, X, Wt, Bias, Y):
    nc = tc.nc
    with (
        tc.tile_pool(name="const", bufs=1) as cpool,
        tc.tile_pool(name="xpool", bufs=3) as xpool,
        tc.tile_pool(name="ypool", bufs=6) as ypool,
        tc.tile_pool(name="ppool", bufs=8, space="PSUM") as ppool,
    ):
        wt_sb = cpool.tile([128, 3, 64], F16)
        nc.sync.dma_start(out=wt_sb[:], in_=Wt)
        b_sb = cpool.tile([128, 1], F32)

        # warm the PE clock-gate while the first X pieces load; the ramp
        # completes after ~8 matmuls of PE activity (373ns cold -> 190ns
        # warm), so 7 is enough -- more would be throwaway full-speed work
        warm = cpool.tile([128, 448], F16)
        nc.vector.memset(warm[:], 1.0)
        pw = ppool.tile([64, 448], F32, name="pw", tag="pt")
        for _ in range(7):
            nc.tensor.matmul(
                pw[:, :],
                warm[:, 0:64],
                warm[:, :],
                start=True,
                stop=True,
            )

        NP = RS // 8  # pairs per parity per slice
        MH = RS // 8  # m-columns per store half
        NS = NB * NSLICE
        slices = [divmod(s, NSLICE) for s in range(NS)]

        def new_xr0():
            # (jd w) merge into one >=6KB descriptor per partition.
            return xpool.tile([128, NJD, WP], F16, name="xr0", tag="xr0")

        def load_pieces(xr0, s, pieces):
            n, t = slices[s]
            jq0 = (NJD - 1) * t
            for a, b in pieces:
                nc.sync.dma_start(
                    out=xr0[:, a:b, :],
                    in_=X[n, :, :, jq0 + a : jq0 + b, :],
                )

        def build_xr1(xr0, pieces):
            # xr1 (the same rows rotated down by two) is built on-chip:
            # SBUF->SBUF costs queue time but no HBM bandwidth.  The copies
            # ride the scalar (qACT) HWDGE ring: the sync ring's single
            # queue cannot move loads AND copies fast enough at startup.
            # Each copy is issued at a program point where its wait is
            # (nearly) satisfied, so the ACT ring never stalls evictions.
            xr1 = xpool.tile([128, NJD - 1, WP], F16, name="xr1", tag="xr1")
            for a, b in pieces:
                b2 = min(b, NJD - 1)
                nc.scalar.dma_start(out=xr1[0:64, a:b2, :], in_=xr0[64:128, a:b2, :])
                lo = max(a - 1, 0)
                nc.scalar.dma_start(
                    out=xr1[64:128, lo : b - 1, :], in_=xr0[0:64, lo + 1 : b, :]
                )
            return xr1

        # slice 0 loads in fine pieces so the first conv matmul can start
        # right after the warm-up instead of after the full-slice DMA
        xr0_t = [None] * NS
        xr1_t = [None] * NS
        xr0_t[0] = new_xr0()
        load_pieces(xr0_t[0], 0, ((0, 3), (3, 6)))
        nc.sync.dma_start(out=b_sb[:], in_=Bias)  # needed at first eviction
        load_pieces(xr0_t[0], 0, ((6, 10), (10, 14), (14, 20), (20, NJD)))
        # slice 0's copies, piece-wise behind the matching load pieces (on
        # the parallel ACT ring, so they don't eat the sync ring's load
        # bandwidth); slice 1's load follows on the now-free sync ring
        xr1_t[0] = build_xr1(xr0_t[0], ((0, 6), (6, 14), (14, 22), (22, NJD)))
        xr0_t[1] = new_xr0()
        load_pieces(xr0_t[1], 1, ((0, NJD),))

        for s in range(NS):
            n, t = slices[s]
            if 1 <= s < NS - 1:
                # prefetch one slice ahead on the sync ring; the wait is
                # already satisfied (buffer freed two slices ago)
                xr0_t[s + 1] = new_xr0()
                load_pieces(xr0_t[s + 1], s + 1, ((0, NJD),))
            xr0, xr1 = xr0_t[s], xr1_t[s]

            # staging for the whole slice: partition group G = 2*pi + ho,
            # free (m_local = 2i + u, w); fp16 halves the store bytes
            ysb = ypool.tile([128, RS // 4, 224], F16, name="ysb", tag="ysb")
            for pi in range(2):
                # store finished m-pieces while later pairs compute; the
                # very last parity drains in finer chunks on the (idle by
                # then) sync ring to shorten the tail after the last matmul
                last = pi and s == NS - 1
                bounds = (0, MH, 20, 24, 2 * MH) if last else (0, MH, 2 * MH)
                for i in range(NP):  # pair (hbl0, hbl0+2), hbl0 = 4i + pi
                    src = xr0 if pi == 0 else xr1
                    jd0 = 2 * i  # local free index of u=0 in xr0/xr1
                    pt = ppool.tile([64, 2, 224], F32, name="pt", tag="pt")
                    for sh in range(3):
                        nc.tensor.matmul(
                            pt[:, :, :],
                            wt_sb[:, sh, :],
                            src[:, jd0 : jd0 + 2, sh : sh + 224],
                            start=(sh == 0),
                            stop=(sh == 2),
                        )
                    # G = 2*pi + ho == (partition base 64*pi + 32*ho)/32, so
                    # one 64-wide op per pair covers both ho groups
                    dst = ysb[64 * pi : 64 * (pi + 1), 2 * i : 2 * i + 2, :]
                    if i % 2 == 0:
                        nc.scalar.activation(
                            dst,
                            pt[:, :, :],
                            AF.Identity,
                            bias=b_sb[64 * pi : 64 * (pi + 1), :],
                        )
                    else:
                        nc.vector.tensor_scalar_add(
                            dst, pt[:, :, :], b_sb[64 * pi : 64 * (pi + 1), :]
                        )
                    if pi == 1 and i == 12 and s + 1 < NS:
                        # next slice's rotated copy, placed right after this
                        # slice's last scalar eviction: the source load
                        # landed mid-slice, so the ACT-ring wait is already
                        # satisfied and no eviction queues behind it
                        xr1_t[s + 1] = build_xr1(xr0_t[s + 1], ((0, NJD),))
                    # (m, w) merge into one >=6KB descriptor per partition
                    # on the (otherwise idle) gpsimd SWDGE: descriptor
                    # generation on the ACT/sync rings would serialize
                    # with the evictions / the X loads.
                    m = 2 * i + 2
                    if m in bounds[1:]:
                        ci = bounds.index(m) - 1
                        mlo, mhi = bounds[ci], m
                        eng = nc.sync if (last and ci >= 1) else nc.gpsimd
                        eng.dma_start(
                            out=Y[n, 2 * pi : 2 * pi + 2, :,
                                  RS // 4 * t + mlo : RS // 4 * t + mhi, :],
                            in_=ysb[64 * pi : 64 * (pi + 1), mlo : mhi, :],
                        )


def build_nc(nb=NB, repeat=1):
    assert nb == NB
    nc = bacc.Bacc("TRN2", target_bir_lowering=False, debug=False)
    # X[n, q, c, jq, w] holds padded row 4*jq + q
    X = nc.dram_tensor("X", [NB, 4, C, NQ, WP], F16, kind="ExternalInput").ap()
    Wt = nc.dram_tensor("Wt", [128, 3, 64], F16, kind="ExternalInput").ap()
    Bias = nc.dram_tensor("bias", [128, 1], F32, kind="ExternalInput").ap()
    # staged fp16 output: Y[n, G, k, m, w] = conv(n, k, 4*m + G, w)
    Y = nc.dram_tensor("Y", [NB, 4, K, H // 4, W], F16, kind="ExternalOutput").ap()
    with tile.TileContext(nc) as tc:
        if repeat == 1:
            conv_body(tc, X, Wt, Bias, Y)
        else:
            with tc.For_i(0, repeat, 1):
                conv_body(tc, X, Wt, Bias, Y)
    nc.compile()
    return nc


def prep_weights(Wf, b):
    """Wt[32*q+c, s, 32*ho+k] = W[k, c, q-ho, s] (0 outside 0<=r<3)."""
    Wf = np.asarray(Wf, np.float32)
    Wt = np.zeros((128, 3, 64), np.float32)
    for q in range(4):
        for ho in range(2):
            r = q - ho
            if 0 <= r <= 2:
                Wt[32 * q : 32 * q + 32, :, 32 * ho : 32 * ho + 32] = Wf[
                    :, :, r, :
                ].transpose(1, 2, 0)
    bias = np.tile(np.asarray(b, np.float32), 4).reshape(128, 1)
    return Wt.astype(_NP16), bias


def pad_input(X):
    """Pad to 228x226 and pre-rotate rows: out[n, q, c, jd, w] = row 4*jd + q."""
    X = np.ascontiguousarray(X, np.float32)
    Xp = np.zeros((X.shape[0], C, H + 4, WP), _NP16)
    Xp[:, :, 1 : H + 1, 1 : W + 1] = X
    Xr = Xp.reshape(X.shape[0], C, NQ, 4, WP).transpose(0, 3, 1, 2, 4)
    return np.ascontiguousarray(Xr)


_NC = None


def _get_nc():
    global _NC
    if _NC is None:
        _NC = build_nc(NB)
    return _NC


def kernel(X, W, b, _trace=False):
    Xp = pad_input(X)
    Wt, bias = prep_weights(W, b)
    nc = _get_nc()
    in_maps = [
        {"X": Xp[NB * c : NB * (c + 1)], "Wt": Wt, "bias": bias} for c in range(NCORES)
    ]
    res = run_bass_kernel_spmd(nc, in_maps, list(range(NCORES)), trace=_trace)
    # un-interleave the staged layout: Y[n, k, 4*m + G, w] = staged[n, G, k, m, w]
    staged = np.concatenate([res.results[c]["Y"] for c in range(NCORES)], axis=0)
    out = np.ascontiguousarray(
        staged.transpose(0, 2, 3, 1, 4).reshape(NCORES * NB, 32, 224, 224),
        dtype=np.float32,
    )
    if _trace:
        return out, res
    return out



# revision 20
# speedup vs baseline: 1.2918x; 1.2918x over previous
"""Conv2d(32->32, 3x3, stride 1, pad 1) on X[32,32,224,224] fp32, data-parallel
over 8 NeuronCores (4 images per core).

Per-core algorithm ("full-K row-rotated")
-----------------------------------------
The conv is computed as full-array PE matmuls with contraction K = 128 =
(q in 0..3 row-taps) x (c = 32 input channels), M = 64 = (ho in 0..1) x
(k = 32 output channels), N = 448 = (u in 0..1 row-pairs) x (w in 0..223),
in fp16 (1 column/cycle, warms the PE clock gate; fp32 runs 4x slower and
float32r runs cold at 1.2 GHz and overlaps poorly).

X (host-padded to 226 wide, host-cast to fp16, host-rotated so row 4*jd + q
sits at partition group q) is DMA'd per H-slice as Xr0; a second copy Xr1,
rotated down by two rows (j = 4*jd + q + 2), is built on-chip by two
SBUF->SBUF partition-remap DMAs per xr0 half (engine time but no HBM
bandwidth, which the startup is short on).  For an output pair starting at
even hb0, the 4 input rows sit at partition group q, one free offset -- so
one matmul contracts all 4 row-taps at once; even pairs read Xr0, odd
pairs Xr1.  One PSUM accumulation group = 3 matmuls (s = column
shift of the rhs into the padded row).  Per core: 672 matmuls of
[128,64]x[128,448].

lhsT[32*q+c, s][32*ho+k] = W[k, c, q-ho, s] (zero outside 0<=r<3), so both
output rows of an hb are produced per matmul.  Bias is fused into the
PSUM->SBUF eviction (ScalarE/VectorE alternating), which also casts to fp16.
Work is H-sliced (112 output rows) for SBUF fit and load/compute overlap.

Y leaves the device in the staged layout [n, G, k, m, w] (fp16) where the
output row h = 4*m + G -- each store is then one >=6KB descriptor per
partition instead of 896B per (k, m) row, which otherwise saturates all 16
DMA queues with descriptor overhead.  The host un-interleaves + casts fp32.
"""

import sys

import numpy as np

try:
    import concourse.bass as bass  # noqa: F401
except ImportError:  # pragma: no cover
    sys.path.insert(0, "/opt/trn_rl_repo")

import ml_dtypes
import concourse.mybir as mybir
import concourse.tile as tile
from concourse import bacc
from concourse.bass_utils import run_bass_kernel_spmd

NCORES = 8
NB = 4  # images per core
C = 32
K = 32
H = 224
W = 224
WP = 226  # padded width
NQ = 57  # row-quads in the host-rotated layout (228 padded rows / 4)
RS = 112  # output rows per slice
NSLICE = H // RS
NJD = RS // 4 + 1  # row-quads per rotated slice tile
F32 = mybir.dt.float32
F16 = mybir.dt.float16
AF = mybir.ActivationFunctionType
_NP16 = np.float16


def set_dtype(name):
    """'fp16' (default) or 'bf16' for the matmul operand precision."""
    global F16, _NP16, _NC
    if name == "bf16":
        F16, _NP16 = mybir.dt.bfloat16, ml_dtypes.bfloat16
    else:
        F16, _NP16 = mybir.dt.float16, np.float16
    _NC = None


def conv_body(tc, X, Wt, Bias, Y):
    nc = tc.nc
    with (
        tc.tile_pool(name="const", bufs=1) as cpool,
        tc.tile_pool(name="xpool", bufs=3) as xpool,
        tc.tile_pool(name="ypool", bufs=6) as ypool,
        tc.tile_pool(name="ppool", bufs=8, space="PSUM") as ppool,
    ):
        wt_sb = cpool.tile([128, 3, 64], F16)
        b_sb = cpool.tile([128, 1], F32)
        # bias rides the gpsimd SWDGE and is issued first: its real job is
        # warming the SWDGE pipeline (~4us from first descriptor to first
        # packet) so the slice-0 copies below start moving early
        nc.gpsimd.dma_start(out=b_sb[:], in_=Bias)
        nc.sync.dma_start(out=wt_sb[:], in_=Wt)

        # warm the PE clock-gate while the first X pieces load; the DVFS
        # ramp completes after ~8-15 matmuls of PE activity (373ns cold ->
        # 190ns warm), and the first load piece lands ~11.3us, so 10 warm
        # matmuls hand off without a de-ramping idle gap
        warm = cpool.tile([128, 448], F16)
        nc.vector.memset(warm[:], 1.0)
        pw = ppool.tile([64, 448], F32, name="pw", tag="pt")
        for _ in range(10):
            nc.tensor.matmul(
                pw[:, :],
                warm[:, 0:64],
                warm[:, :],
                start=True,
                stop=True,
            )

        NP = RS // 8  # pairs per parity per slice
        MH = RS // 8  # m-columns per store half
        NS = NB * NSLICE
        slices = [divmod(s, NSLICE) for s in range(NS)]

        def new_xr0():
            # (jd w) merge into one >=6KB descriptor per partition.
            return xpool.tile([128, NJD, WP], F16, name="xr0", tag="xr0")

        def load_pieces(xr0, s, pieces, eng=None):
            n, t = slices[s]
            jq0 = (NJD - 1) * t
            for a, b in pieces:
                (eng or nc.sync).dma_start(
                    out=xr0[:, a:b, :],
                    in_=X[n, :, :, jq0 + a : jq0 + b, :],
                )

        def new_xr1():
            return xpool.tile([128, NJD - 1, WP], F16, name="xr1", tag="xr1")

        def build_half(xr1, xr0, a, b, eng):
            # xr1 (the same rows rotated down by two) is built on-chip:
            # SBUF->SBUF costs queue time but no HBM bandwidth, which the
            # startup (this tile + next tile's prefetch) is short on.
            b2 = min(b, NJD - 1)
            eng.dma_start(out=xr1[0:64, a:b2, :], in_=xr0[64:128, a:b2, :])
            lo = max(a - 1, 0)
            eng.dma_start(out=xr1[64:128, lo : b - 1, :], in_=xr0[0:64, lo + 1 : b, :])

        for s in range(NS):
            n, t = slices[s]
            xr0 = new_xr0()
            xr1 = new_xr1()
            if s == 0:
                # slice 0 loads in fine pieces so the first conv matmul
                # starts right after the (shortened) warm-up, rate-matched
                # to the sync ring's ~0.2 MB/us delivery; its rotation
                # copies ride the otherwise-idle gpsimd SWDGE ring, freeing
                # ~8 us of sync-ring time so slice 1's load lands early
                load_pieces(xr0, 0, ((21, NJD),), nc.gpsimd)  # no waits
                load_pieces(xr0, 0, ((0, 4), (4, 9), (9, 14), (14, 21)))
                # the SWDGE queue moves only ~0.125 MB/us (16 engines), so
                # it can't carry the whole 1.6M copy in time: quads 14..20
                # ride the sync ring right behind the load (idle by then),
                # the rest stay on SWDGE
                for a, b, eng in (
                    (0, 6, nc.gpsimd),
                    (6, 14, nc.gpsimd),
                    (14, 21, nc.sync),
                    (21, NJD, nc.gpsimd),
                ):
                    build_half(xr1, xr0, a, b, eng)
            else:
                # split loads: with packet-round-robin across in-flight
                # DMAs, smaller pieces make the earliest quads land sooner.
                # Copies are issued after both loads so a copy's ring-FIFO
                # wait never delays a load's descriptor generation.
                halves = ((0, 14), (14, NJD))
                load_pieces(xr0, s, halves)
                for a, b in halves:
                    build_half(xr1, xr0, a, b, nc.sync)

            # staging for the whole slice: partition group G = 2*pi + ho,
            # free (m_local = 2i + u, w); fp16 halves the store bytes
            ysb = ypool.tile([128, RS // 4, 224], F16, name="ysb", tag="ysb")
            for pi in range(2):
                # store finished m-pieces while later pairs compute; the
                # very last parity drains in finer chunks on the (idle by
                # then) sync ring to shorten the tail after the last matmul
                last = pi and s == NS - 1
                bounds = (0, MH, 20, 24, 2 * MH) if last else (0, MH, 2 * MH)
                for i in range(NP):  # pair (hbl0, hbl0+2), hbl0 = 4i + pi
                    src = xr0 if pi == 0 else xr1
                    jd0 = 2 * i  # local free index of u=0 in xr0/xr1
                    pt = ppool.tile([64, 2, 224], F32, name="pt", tag="pt")
                    for sh in range(3):
                        nc.tensor.matmul(
                            pt[:, :, :],
                            wt_sb[:, sh, :],
                            src[:, jd0 : jd0 + 2, sh : sh + 224],
                            start=(sh == 0),
                            stop=(sh == 2),
                        )
                    # G = 2*pi + ho == (partition base 64*pi + 32*ho)/32, so
                    # one 64-wide op per pair covers both ho groups
                    dst = ysb[64 * pi : 64 * (pi + 1), 2 * i : 2 * i + 2, :]
                    if i % 2 == 0:
                        nc.scalar.activation(
                            dst,
                            pt[:, :, :],
                            AF.Identity,
                            bias=b_sb[64 * pi : 64 * (pi + 1), :],
                        )
                    else:
                        nc.vector.tensor_scalar_add(
                            dst, pt[:, :, :], b_sb[64 * pi : 64 * (pi + 1), :]
                        )

                    # (m, w) merge into one >=6KB descriptor per partition
                    # on the (otherwise idle) gpsimd SWDGE: descriptor
                    # generation on the ACT/sync rings would serialize
                    # with the evictions / the X loads.
                    m = 2 * i + 2
                    if m in bounds[1:]:
                        ci = bounds.index(m) - 1
                        mlo, mhi = bounds[ci], m
                        eng = nc.sync if (last and ci >= 1) else nc.gpsimd
                        eng.dma_start(
                            out=Y[n, 2 * pi : 2 * pi + 2, :,
                                  RS // 4 * t + mlo : RS // 4 * t + mhi, :],
                            in_=ysb[64 * pi : 64 * (pi + 1), mlo : mhi, :],
                        )


def build_nc(nb=NB, repeat=1):
    assert nb == NB
    nc = bacc.Bacc("TRN2", target_bir_lowering=False, debug=False)
    # X[n, q, c, jq, w] holds padded row 4*jq + q
    X = nc.dram_tensor("X", [NB, 4, C, NQ, WP], F16, kind="ExternalInput").ap()
    Wt = nc.dram_tensor("Wt", [128, 3, 64], F16, kind="ExternalInput").ap()
    Bias = nc.dram_tensor("bias", [128, 1], F32, kind="ExternalInput").ap()
    # staged fp16 output: Y[n, G, k, m, w] = conv(n, k, 4*m + G, w)
    Y = nc.dram_tensor("Y", [NB, 4, K, H // 4, W], F16, kind="ExternalOutput").ap()
    with tile.TileContext(nc) as tc:
        if repeat == 1:
            conv_body(tc, X, Wt, Bias, Y)
        else:
            with tc.For_i(0, repeat, 1):
                conv_body(tc, X, Wt, Bias, Y)
    nc.compile()
    return nc


def prep_weights(Wf, b):
    """Wt[32*q+c, s, 32*ho+k] = W[k, c, q-ho, s] (0 outside 0<=r<3)."""
    Wf = np.asarray(Wf, np.float32)
    Wt = np.zeros((128, 3, 64), np.float32)
    for q in range(4):
        for ho in range(2):
            r = q - ho
            if 0 <= r <= 2:
                Wt[32 * q : 32 * q + 32, :, 32 * ho : 32 * ho + 32] = Wf[
                    :, :, r, :
                ].transpose(1, 2, 0)
    bias = np.tile(np.asarray(b, np.float32), 4).reshape(128, 1)
    return Wt.astype(_NP16), bias


def pad_input(X):
    """Pad to 228x226 and pre-rotate rows: out[n, q, c, jd, w] = row 4*jd + q."""
    X = np.ascontiguousarray(X, np.float32)
    Xp = np.zeros((X.shape[0], C, H + 4, WP), _NP16)
    Xp[:, :, 1 : H + 1, 1 : W + 1] = X
    Xr = Xp.reshape(X.shape[0], C, NQ, 4, WP).transpose(0, 3, 1, 2, 4)
    return np.ascontiguousarray(Xr)


_NC = None


def _get_nc():
    global _NC
    if _NC is None:
        _NC = build_nc(NB)
    return _NC


def kernel(X, W, b, _trace=False):
    Xp = pad_input(X)
    Wt, bias = prep_weights(W, b)
    nc = _get_nc()
    in_maps = [
        {"X": Xp[NB * c : NB * (c + 1)], "Wt": Wt, "bias": bias} for c in range(NCORES)
    ]
    res = run_bass_kernel_spmd(nc, in_maps, list(range(NCORES)), trace=_trace)
    # un-interleave the staged layout: Y[n, k, 4*m + G, w] = staged[n, G, k, m, w]
    staged = np.concatenate([res.results[c]["Y"] for c in range(NCORES)], axis=0)
    out = np.ascontiguousarray(
        staged.transpose(0, 2, 3, 1, 4).reshape(NCORES * NB, 32, 224, 224),
        dtype=np.float32,
    )
    if _trace:
        return out, res
    return out

